# revision 30
# baseline (speedup 1.0000x reference)
"""ChildSum TreeLSTM on 8 Trainium2 NeuronCores.

Sharding: tensor-parallel over hidden dim H=1024 — core j owns output-feature
slice [128j, 128j+128) of every gate matrix. Each core gathers all B*N token
embeddings (bf16, feature-major via transposing dma_gather), computes its
slice of the x-side projections, then sweeps the tree bottom-up. After each
level the 8 h-slices are AllGathered (bf16) so every core has the full-K
child h for the next level's matmuls. c stays sliced; the root c slice and a
partial (K-sliced) logits matmul are returned per core and assembled on host.
"""

import contextlib
import os
import numpy as np
import ml_dtypes

import concourse.bass as bass
import concourse.bacc as bacc
import concourse.mybir as mybir
import concourse.tile as tile
from concourse.bass_utils import run_bass_kernel_spmd

BF16 = mybir.dt.bfloat16
F32 = mybir.dt.float32
I16 = mybir.dt.int16
SIG = mybir.ActivationFunctionType.Sigmoid
TANH = mybir.ActivationFunctionType.Tanh

B = 32          # trees
NN = 63         # nodes per tree (perfect binary, heap order)
E = H = 1024
V = 32000
L = 5
NCORES = 8
P = 128
KT = H // P     # 8 K-tiles of 128

# column layout: per level, col = tree*m + node  (m = 2**d nodes/tree)
LVL_ROWS = {d: B * (2 ** d) for d in range(6)}          # cols at level d
INT_OFF = {4: 0, 3: 512, 2: 768, 1: 896, 0: 960}        # into internal block
LEAF_COLS = 1024                                         # level 5: 32*32
INT_COLS = 1024                                          # 992 real + 32 pad

AG_GROUP = [list(range(NCORES))]

_CACHE: dict = {}


def _wrap_idxs(idx_flat: np.ndarray, pad_to: int) -> np.ndarray:
    """SWDGE gather index layout: unwrapped[i] = wrapped[i%16, i//16],
    16-partition block replicated to 128 partitions."""
    assert pad_to % 128 == 0 and idx_flat.size <= pad_to
    idx = np.zeros((pad_to,), np.int64)
    idx[: idx_flat.size] = idx_flat
    assert idx.max() < 2 ** 15
    w16 = idx.reshape(pad_to // 16, 16).T.astype(np.int16)  # [16, n/16]
    return np.ascontiguousarray(np.tile(w16, (8, 1)))        # [128, n/16]


def _build():
    """Build + compile the (single SPMD) bass program. Cached."""
    if "nc" in _CACHE:
        return _CACHE["nc"]

    dbg = bool(int(os.environ.get("KERNEL_DEBUG", "0")))
    nc = bacc.Bacc("TRN2", target_bir_lowering=False, debug=False,
                   num_devices=NCORES)
    dbg_outs = {}

    def dbg_tap(name, ap, shape, dtype):
        if not dbg:
            return
        t = nc.dram_tensor(f"dbg_{name}", shape, dtype, kind="ExternalOutput")
        nc.sync.dma_start(t.ap(), ap)
        dbg_outs[name] = t

    # ---------------- I/O ----------------
    emb = nc.dram_tensor("emb", [V, E], BF16, kind="ExternalInput")
    idx_leaf_d = nc.dram_tensor("idx_leaf", [P, LEAF_COLS // 16], I16,
                                kind="ExternalInput")
    idx_int_d = nc.dram_tensor("idx_int", [P, INT_COLS // 16], I16,
                               kind="ExternalInput")
    # x-side W.T slices, gate order 0=i, 1=f, 2=u:  wx[g, kk, m] = Wg.T[kk, j*128+m]
    wx_d = nc.dram_tensor("wx", [3, H, P], BF16, kind="ExternalInput")
    # h-side W.T slices, gate order 0=i, 1=u, 2=f
    wh_d = nc.dram_tensor("wh", [3, H, P], BF16, kind="ExternalInput")
    wout_d = nc.dram_tensor("wout", [P, L], BF16, kind="ExternalInput")
    bias_d = nc.dram_tensor("bias", [P, 3], F32, kind="ExternalInput")  # i,f,u
    c_out_d = nc.dram_tensor("c_out", [P, B], F32, kind="ExternalOutput")
    log_out_d = nc.dram_tensor("log_out", [B, L], F32, kind="ExternalOutput")

    with tile.TileContext(nc) as tc:
        ctx = contextlib.ExitStack()
        with ctx:
            ps = ctx.enter_context(
                tc.tile_pool(name="ps", bufs=7, space=bass.MemorySpace.PSUM))
            pslog = ctx.enter_context(
                tc.tile_pool(name="pslog", bufs=1, space=bass.MemorySpace.PSUM))
            wk = ctx.enter_context(tc.tile_pool(name="wk", bufs=14))
            dram = ctx.enter_context(
                tc.tile_pool(name="dram", bufs=1, space="DRAM"))
            sb = ctx.enter_context(tc.tile_pool(name="sb", bufs=1))

            wkn = [0]

            def wkt(rows, dtype=F32, tag="wk"):
                wkn[0] += 1
                return wk.tile([P, rows], dtype, tag=tag, name=f"wk{wkn[0]}",
                               padded_shape=[P, 512])

            # ---------------- index loads (SP ring — first in FIFO) -------
            idx_leaf = sb.tile([P, LEAF_COLS // 16], I16, tag="idxl")
            idx_int = sb.tile([P, INT_COLS // 16], I16, tag="idxi")
            nc.sync.dma_start(idx_leaf[:], idx_leaf_d[:])
            nc.sync.dma_start(idx_int[:], idx_int_d[:])

            # ---------------- embedding gathers (feature-major x.T) -------
            xT_leaf_t = sb.tile([P, KT, LEAF_COLS], BF16, tag="xtl")
            xT_int_t = sb.tile([P, KT, INT_COLS], BF16, tag="xti")
            nc.gpsimd.dma_gather(
                out_ap=xT_leaf_t[:], in_ap=emb.ap(), idxs_ap=idx_leaf[:],
                num_idxs=LEAF_COLS, num_idxs_reg=LEAF_COLS,
                elem_size=E, transpose=True, single_packet=False)
            nc.gpsimd.dma_gather(
                out_ap=xT_int_t[:], in_ap=emb.ap(), idxs_ap=idx_int[:],
                num_idxs=INT_COLS, num_idxs_reg=INT_COLS,
                elem_size=E, transpose=True, single_packet=False)

            # ---------------- constants in (ACT ring) ---------------------
            wx = sb.tile([P, 3, KT, P], BF16, tag="wx")
            wh = sb.tile([P, 3, KT, P], BF16, tag="wh")
            wout = sb.tile([P, L], BF16, tag="wout")
            bias = sb.tile([P, 3], F32, tag="bias")
            nc.scalar.dma_start(wx[:], wx_d.ap().rearrange("g (k p) m -> p g k m", p=P))
            nc.scalar.dma_start(wh[:], wh_d.ap().rearrange("g (k p) m -> p g k m", p=P))
            nc.scalar.dma_start(wout[:], wout_d[:])
            nc.scalar.dma_start(bias[:], bias_d[:])
            b_i = bias[:, 0:1]
            b_f = bias[:, 1:2]
            b_u = bias[:, 2:3]

            # ---------------- leaf level (x-side gates -> h,c) ------------
            xi_int = sb.tile([P, INT_COLS], F32, tag="xii")
            xf_int = sb.tile([P, INT_COLS], F32, tag="xfi")
            xu_int = sb.tile([P, INT_COLS], F32, tag="xui")
            x_dst = {0: xi_int, 1: xf_int, 2: xu_int}

            cT5 = sb.tile([P, LEAF_COLS], F32, tag="c5")       # leaf c slice
            h5_bf = sb.tile([P, LEAF_COLS], BF16, tag="h5")    # leaf h slice
            ag_in5 = dram.tile([P, LEAF_COLS], BF16, tag="agi5")
            ag_out5 = dram.tile([NCORES, P, LEAF_COLS], BF16, tag="ago5",
                                addr_space="Shared")

            for ch in range(2):
                cs = slice(ch * 512, (ch + 1) * 512)
                pg = []
                for g in range(3):
                    pt = ps.tile([P, 512], F32, tag="ps", name=f"psl{ch}{g}")
                    for k in range(KT):
                        nc.tensor.matmul(pt[:], wx[:, g, k, :],
                                         xT_leaf_t[:, k, cs],
                                         start=(k == 0), stop=(k == KT - 1))
                    pg.append(pt)
                i_t = wkt(512)
                o_t = wkt(512)
                u_t = wkt(512)
                nc.scalar.activation(i_t[:], pg[0][:], SIG, bias=b_i)
                nc.scalar.activation(o_t[:], pg[1][:], SIG, bias=b_f)
                nc.scalar.activation(u_t[:], pg[2][:], TANH, bias=b_u)
                nc.vector.tensor_mul(cT5[:, cs], i_t[:], u_t[:])
                th_t = wkt(512)
                nc.scalar.activation(th_t[:], cT5[:, cs], TANH)
                nc.vector.tensor_mul(h5_bf[:, cs], o_t[:], th_t[:])
                # stage this chunk for the AllGather right away
                nc.sync.dma_start(ag_in5[:, cs], h5_bf[:, cs])

            dbg_tap("xl0", xT_leaf_t[:, :, :512], [P, KT, 512], BF16)
            dbg_tap("xl1", xT_leaf_t[:, :, 512:], [P, KT, 512], BF16)
            dbg_tap("h5", h5_bf[:], [P, LEAF_COLS], BF16)
            dbg_tap("c5", cT5[:], [P, LEAF_COLS], F32)

            nc.gpsimd.collective_compute(
                "AllGather", mybir.AluOpType.bypass, replica_groups=AG_GROUP,
                ins=[ag_in5.opt()], outs=[ag_out5.opt()])

            # per-rank child-h tiles: K-tile r comes from rank r
            hc = [sb.tile([P, LEAF_COLS], BF16, name=f"hc5_{r}", tag=f"hc5_{r}")
                  for r in range(NCORES)]
            for r in range(NCORES):
                nc.sync.dma_start(hc[r][:], ag_out5[r])

            # ---------------- internal x-projections (overlap the AG) -----
            for ch in range(2):
                cs = slice(ch * 512, (ch + 1) * 512)
                for g in range(3):
                    pt = ps.tile([P, 512], F32, tag="ps", name=f"psi{ch}{g}")
                    for k in range(KT):
                        nc.tensor.matmul(pt[:], wx[:, g, k, :],
                                         xT_int_t[:, k, cs],
                                         start=(k == 0), stop=(k == KT - 1))
                    nc.vector.tensor_copy(x_dst[g][:, cs], pt[:])

            cT_child = cT5
            dbg_tap("xi", xi_int[:], [P, INT_COLS], F32)
            dbg_tap("xf", xf_int[:], [P, INT_COLS], F32)
            dbg_tap("hc0", hc[0][:], [P, LEAF_COLS], BF16)
            dbg_tap("hc7", hc[7][:], [P, LEAF_COLS], BF16)

            # ---------------- tree sweep d=4..0 ---------------------------
            for d in range(4, -1, -1):
                rows = LVL_ROWS[d]           # nodes this level (cols)
                crows = 2 * rows             # children cols
                off = INT_OFF[d]
                xi_l = xi_int[:, off:off + rows]
                xf_l = xf_int[:, off:off + rows]
                xu_l = xu_int[:, off:off + rows]

                # per-child f projections P_all = Wfh_slice @ child_h
                nfc = (crows + 511) // 512
                fw = min(512, crows)
                p_f = []
                for fc in range(nfc):
                    fs = slice(fc * fw, (fc + 1) * fw)
                    pt = ps.tile([P, fw], F32, tag="ps", name=f"psf{d}{fc}",
                                 padded_shape=[P, 512])
                    for k in range(KT):
                        nc.tensor.matmul(pt[:], wh[:, 2, k, :], hc[k][:, fs],
                                         start=(k == 0), stop=(k == KT - 1))
                    p_f.append(pt)

                # hsum per K-tile on DVE (overlaps the f matmuls on PE)
                hsum = [sb.tile([P, rows], BF16, name=f"hs{d}_{k}",
                                tag=f"hs{d}_{k}") for k in range(KT)]
                for k in range(KT):
                    nc.vector.tensor_add(hsum[k][:], hc[k][:, 0::2],
                                         hc[k][:, 1::2])

                p_i = ps.tile([P, rows], F32, tag="ps", name=f"psi{d}",
                              padded_shape=[P, 512])
                p_u = ps.tile([P, rows], F32, tag="ps", name=f"psu{d}",
                              padded_shape=[P, 512])
                for k in range(KT):
                    nc.tensor.matmul(p_i[:], wh[:, 0, k, :], hsum[k][:],
                                     start=(k == 0), stop=(k == KT - 1))
                for k in range(KT):
                    nc.tensor.matmul(p_u[:], wh[:, 1, k, :], hsum[k][:],
                                     start=(k == 0), stop=(k == KT - 1))

                # ---- elementwise ----
                i_t = wkt(rows)
                u_t = wkt(rows)
                ti = wkt(rows)
                tu = wkt(rows)
                nc.vector.tensor_add(ti[:], p_i[:], xi_l)
                nc.scalar.activation(i_t[:], ti[:], SIG, bias=b_i)
                nc.vector.tensor_add(tu[:], p_u[:], xu_l)
                nc.scalar.activation(u_t[:], tu[:], TANH, bias=b_u)

                c_new = sb.tile([P, rows], F32, tag=f"c{d}")
                nc.vector.tensor_mul(c_new[:], i_t[:], u_t[:])

                h_bf = sb.tile([P, rows], BF16, tag=f"h{d}")
                if d > 0:
                    ag_in = dram.tile([P, rows], BF16, tag=f"agi{d}")
                    ag_out = dram.tile([NCORES, P, rows], BF16, tag=f"ago{d}",
                                       addr_space="Shared")

                o_t = wkt(rows)
                th = wkt(rows)
                for fc in range(nfc):
                    # node range covered by this f-chunk
                    n0 = fc * (fw // 2)
                    n1 = n0 + (fw // 2)
                    ns = slice(n0, n1)
                    pa = p_f[fc]
                    w0 = wkt(fw // 2)
                    w1 = wkt(fw // 2)
                    nc.vector.tensor_add(w0[:], pa[:, 0::2], xf_l[:, ns])
                    nc.vector.tensor_add(w1[:], pa[:, 1::2], xf_l[:, ns])
                    to = wkt(fw // 2)
                    nc.vector.tensor_add(to[:], w0[:], pa[:, 1::2])
                    nc.scalar.activation(o_t[:, ns], to[:], SIG, bias=b_f)
                    f0 = wkt(fw // 2)
                    f1 = wkt(fw // 2)
                    nc.scalar.activation(f0[:], w0[:], SIG, bias=b_f)
                    nc.scalar.activation(f1[:], w1[:], SIG, bias=b_f)
                    fc0 = wkt(fw // 2)
                    fc1 = wkt(fw // 2)
                    c0s = slice(fc * fw, (fc + 1) * fw, 2)
                    c1s = slice(fc * fw + 1, (fc + 1) * fw, 2)
                    nc.vector.tensor_mul(fc0[:], f0[:], cT_child[:, c0s])
                    nc.vector.tensor_mul(fc1[:], f1[:], cT_child[:, c1s])
                    nc.vector.tensor_add(c_new[:, ns], c_new[:, ns], fc0[:])
                    nc.vector.tensor_add(c_new[:, ns], c_new[:, ns], fc1[:])
                    nc.scalar.activation(th[:, ns], c_new[:, ns], TANH)
                    nc.vector.tensor_mul(h_bf[:, ns], o_t[:, ns], th[:, ns])
                    if d > 0:
                        nc.sync.dma_start(ag_in[:, ns], h_bf[:, ns])

                dbg_tap(f"c{d}", c_new[:], [P, rows], F32)
                dbg_tap(f"hb{d}", h_bf[:], [P, rows], BF16)

                if d > 0:
                    nc.gpsimd.collective_compute(
                        "AllGather", mybir.AluOpType.bypass,
                        replica_groups=AG_GROUP,
                        ins=[ag_in.opt()], outs=[ag_out.opt()])
                    hc = [sb.tile([P, rows], BF16, name=f"hc{d}_{r}",
                                  tag=f"hc{d}_{r}") for r in range(NCORES)]
                    for r in range(NCORES):
                        nc.sync.dma_start(hc[r][:], ag_out[r])
                    cT_child = c_new
                else:
                    # outputs: c slice + partial logits
                    nc.sync.dma_start(c_out_d[:], c_new[:])
                    pl = pslog.tile([B, L], F32, tag="pl")
                    nc.tensor.matmul(pl[:], h_bf[:], wout[:],
                                     start=True, stop=True)
                    lsb = sb.tile([B, L], F32, tag="lsb")
                    nc.vector.tensor_copy(lsb[:], pl[:])
                    nc.sync.dma_start(log_out_d[:], lsb[:])

    nc.compile()
    _CACHE["nc"] = nc
    return nc


def _prep_inputs(tokens, embed, Wix, bix, Wih, bih, Wfx, bfx, Wfh, bfh,
                 Wux, bux, Wuh, buh, Wout, bout):
    """Host-side shard prep: returns per-core in_maps."""
    bf = ml_dtypes.bfloat16
    tokens = np.asarray(tokens)
    emb_bf = np.ascontiguousarray(np.asarray(embed, np.float32).astype(bf))

    leaf_tok = tokens[:, 31:63].reshape(-1)                   # t-major
    int_tok = np.concatenate(
        [tokens[:, 2 ** d - 1: 2 ** (d + 1) - 1].reshape(-1)
         for d in (4, 3, 2, 1, 0)])
    idx_leaf = _wrap_idxs(leaf_tok, LEAF_COLS)
    idx_int = _wrap_idxs(int_tok, INT_COLS)

    bias_i = (np.asarray(bix) + np.asarray(bih)).astype(np.float32)
    bias_f = (np.asarray(bfx) + np.asarray(bfh)).astype(np.float32)
    bias_u = (np.asarray(bux) + np.asarray(buh)).astype(np.float32)

    WxT = [np.asarray(Wg, np.float32).T for Wg in (Wix, Wfx, Wux)]  # [E, H]
    WhT = [np.asarray(Wg, np.float32).T for Wg in (Wih, Wuh, Wfh)]
    WoutT = np.asarray(Wout, np.float32).T                           # [H, L]

    in_maps = []
    for j in range(NCORES):
        sl = slice(j * P, (j + 1) * P)
        wx = np.ascontiguousarray(
            np.stack([W[:, sl] for W in WxT]).astype(bf))     # [3, H, P]
        wh = np.ascontiguousarray(
            np.stack([W[:, sl] for W in WhT]).astype(bf))
        wout = np.ascontiguousarray(WoutT[sl, :].astype(bf))  # [P, L]
        bias = np.ascontiguousarray(
            np.stack([bias_i[sl], bias_f[sl], bias_u[sl]], axis=1))  # [P,3]
        in_maps.append({
            "emb": emb_bf,
            "idx_leaf": idx_leaf,
            "idx_int": idx_int,
            "wx": wx,
            "wh": wh,
            "wout": wout,
            "bias": bias.astype(np.float32),
        })
    return in_maps


def kernel(tokens, embed, Wix, bix, Wih, bih, Wfx, bfx, Wfh, bfh,
           Wux, bux, Wuh, buh, Wout, bout):
    nc = _build()
    in_maps = _prep_inputs(tokens, embed, Wix, bix, Wih, bih, Wfx, bfx,
                           Wfh, bfh, Wux, bux, Wuh, buh, Wout, bout)
    trace = bool(int(os.environ.get("KERNEL_TRACE", "0")))
    br = run_bass_kernel_spmd(nc, in_maps, core_ids=list(range(NCORES)),
                              trace=trace)
    kernel.last_results = br

    c_full = np.empty((B, H), np.float32)
    logits = np.zeros((B, L), np.float64)
    for j, res in enumerate(br.results):
        c_full[:, j * P:(j + 1) * P] = res["c_out"].T
        logits += res["log_out"].astype(np.float64)
    logits = (logits + np.asarray(bout, np.float64)).astype(np.float32)
    return c_full, logits
